# revision 1
# baseline (speedup 1.0000x reference)
"""Trainium2 Bass kernel for nn_CustomizedLinear (masked pathway linear).

out[b, p*768+e] = sum_d x[b,d] * (weight*mask.T)[p,d] * G[d,e] + bias[p]
with B=64, P=256, D=2000, E=768.

Sharding: tensor-parallel over the pathway dim P — 32 pathways per core on
8 cores; x and gene_embedding replicated.

Per-core compute: for each pathway p, scale x columns by wm[p] (DVE
broadcast multiply) and matmul with G. The TensorE matmul costs N cycles
per instruction regardless of K/M, so we pack 2 pathways x 64 batch rows
into the stationary operand (M=128) and stream G in N=384 chunks,
accumulating 16 k-tiles of 125 into PSUM. All matmul operands are
float32r (fp32 with 11-bit mantissa, 1 cycle/row vs 4 for fp32; rel err
~1.5e-4 at this depth). Input/output DMAs are split across both HWDGE
rings (SP + Activation) so G streaming does not starve the PE.
"""
import sys

sys.path.insert(0, "/opt/trn_rl_repo")

import numpy as np
from contextlib import ExitStack

import concourse.bacc as bacc
import concourse.tile as tile
import concourse.mybir as mybir
from concourse.bass_utils import run_bass_kernel_spmd

F32 = mybir.dt.float32
F32R = mybir.dt.float32r

N_CORES = 8
B = 64          # batch
D = 2000        # genes (contraction)
E = 768         # embedding
P_TOT = 256     # pathways
P_CORE = P_TOT // N_CORES        # 32 pathways per core
KT = 16                          # k-tiles
KP = D // KT                     # 125 rows per k-tile
NCH = 2                          # N chunks per pair
NC_N = E // NCH                  # 384


def _build_program(repeat=1, group_sizes=(8, 4, 4, 4, 4, 4, 2, 2),
                   split_rings=True, psum_bufs=8,
                   strip_bufs=6, g_chunks=(1,) * KT, g_rings=(0, 0, 1)):
    assert sum(group_sizes) == P_CORE
    nc = bacc.Bacc()
    # x/w/m arrive host-permuted k-major: per k-tile a contiguous block
    # [x_k (B) | w_k (P_CORE) | m_k (P_CORE)]; a small head DMA (k=0,1)
    # lets the strip pipeline start before the bulk load finishes
    BLK = B + 2 * P_CORE
    XWM_W = KT * BLK
    HEAD = 2
    xwm_d = nc.declare_dram_parameter("xwm", [KP, XWM_W], F32, isOutput=False)
    g_d = nc.declare_dram_parameter("g", [D, E], F32, isOutput=False)
    bias_d = nc.declare_dram_parameter("bias", [2 * B, P_CORE // 2], F32,
                                       isOutput=False)
    out_d = nc.declare_dram_parameter("out", [B, P_CORE * E], F32, isOutput=True)

    def ring(i):
        if not split_rings:
            return nc.sync
        return nc.sync if i % 2 == 0 else nc.scalar

    with tile.TileContext(nc) as tc, ExitStack() as ctx:
        const = ctx.enter_context(tc.tile_pool(name="const", bufs=1))
        stage = ctx.enter_context(tc.tile_pool(name="stage", bufs=3))
        strips = ctx.enter_context(tc.tile_pool(name="strips", bufs=strip_bufs))
        outs = ctx.enter_context(tc.tile_pool(name="outs", bufs=4))
        psum = ctx.enter_context(
            tc.tile_pool(name="psum", bufs=psum_bufs, space="PSUM"))

        # x/w/m: head (k<HEAD) now; tail spliced into the G stream below
        xwm_h = const.tile([KP, HEAD * BLK], F32)
        nc.scalar.dma_start(out=xwm_h[:], in_=xwm_d[:, :HEAD * BLK])
        bias_t = const.tile([2 * B, P_CORE // 2], F32)
        nc.scalar.dma_start(out=bias_t[:], in_=bias_d[:])
        MID = 10
        xwm_t1 = const.tile([KP, (MID - HEAD) * BLK], F32)
        xwm_t2 = const.tile([KP, (KT - MID) * BLK], F32)

        def blk(k):
            if k < HEAD:
                return xwm_h[:, BLK * k:BLK * (k + 1)]
            if k < MID:
                o = BLK * (k - HEAD)
                return xwm_t1[:, o:o + BLK]
            o = BLK * (k - MID)
            return xwm_t2[:, o:o + BLK]

        x_t, wm_t = [None] * KT, [None] * KT

        def emit_wm(ka, kb):
            for k in range(ka, kb):
                b = blk(k)
                x_t[k] = b[:, :B]
                wm = const.tile([KP, P_CORE], F32, tag=f"wm{k}",
                                name=f"wm{k}")
                nc.vector.tensor_mul(wm[:], b[:, B:B + P_CORE],
                                     b[:, B + P_CORE:])
                wm_t[k] = wm

        emit_wm(0, HEAD)

        # G stream: uneven chunks so the first cast starts early; ring
        # placement per g_rings; casts to f32r on the idle gpsimd engine
        g_view = g_d[:].rearrange("(k d) e -> d k e", k=KT)
        g_r = []
        k0 = 0
        for c, w in enumerate(g_chunks):
            if c == 2:  # xwm tail pt1 after G has a head start
                nc.scalar.dma_start(out=xwm_t1[:],
                                    in_=xwm_d[:, HEAD * BLK:MID * BLK])
                emit_wm(HEAD, MID)
            if c == 4:
                nc.scalar.dma_start(out=xwm_t2[:], in_=xwm_d[:, MID * BLK:])
                emit_wm(MID, KT)
            gs = const.tile([KP, w * E], F32, name=f"gs{c}")
            dst = gs[:].rearrange("d (k e) -> d k e", k=w)
            eng = nc.sync if g_rings[c % len(g_rings)] == 0 else nc.scalar
            eng.dma_start(out=dst, in_=g_view[:, k0:k0 + w, :])
            for j in range(w):
                gr = const.tile([KP, E], F32R, tag=f"g{k0 + j}",
                                name=f"g{k0 + j}")
                cast_eng = nc.vector if k0 + j < 1 else nc.gpsimd
                cast_eng.tensor_copy(gr[:], gs[:, E * j:E * (j + 1)])
                g_r.append(gr)
            k0 += w
        assert k0 == KT


        out_p = out_d[:].rearrange("b (p e) -> p b e", p=P_CORE)  # [32, 64, 768]

        if repeat > 1:
            loop_cm = tc.For_i(0, repeat, 1,
                               hint_engines=(mybir.EngineType.PE,))
            loop_cm.__enter__()

        odma = [0]
        p_start = 0
        for g, gp in enumerate(group_sizes):
            npair = gp // 2
            ps = [psum.tile([2 * B, NC_N], F32, tag="ps", name=f"ps{g}_{i}")
                  for i in range(npair * NCH)]
            for k in range(KT):
                st = strips.tile([KP, gp * B], F32R, tag=f"strip{gp}",
                                 name=f"st{g}_{k}")
                st3 = st[:].rearrange("d (p b) -> d p b", p=gp)
                x_bc = x_t[k][:].unsqueeze(1).broadcast_to([KP, gp, B])
                w_bc = (wm_t[k][:, p_start:p_start + gp]
                        .unsqueeze(2).broadcast_to([KP, gp, B]))
                nc.vector.tensor_mul(st3, x_bc, w_bc)
                for pair in range(npair):
                    lhsT = st[:, 2 * B * pair:2 * B * (pair + 1)]
                    for n in range(NCH):
                        nc.tensor.matmul(
                            ps[NCH * pair + n][:],
                            lhsT,
                            g_r[k][:, NC_N * n:NC_N * (n + 1)],
                            start=(k == 0),
                            stop=(k == KT - 1),
                        )
            for pair in range(npair):
                pg = p_start // 2 + pair       # global pair index 0..15
                last = (g == len(group_sizes) - 1 and pair == npair - 1)
                o = outs.tile([2 * B, E], F32, tag="o", name=f"o{g}_{pair}")
                p0 = 2 * pg
                for n in range(NCH):
                    nc.scalar.activation(
                        o[:, NC_N * n:NC_N * (n + 1)], ps[NCH * pair + n][:],
                        mybir.ActivationFunctionType.Identity,
                        bias=bias_t[:, pg:pg + 1],
                    )
                    if last:
                        dst = out_p[p0:p0 + 2, :, NC_N * n:NC_N * (n + 1)]
                        ring(odma[0]).dma_start(
                            out=dst, in_=o[:, NC_N * n:NC_N * (n + 1)])
                        odma[0] += 1
                if not last:
                    dst = out_p[p0:p0 + 2, :, :]
                    ring(odma[0]).dma_start(out=dst, in_=o[:])
                    odma[0] += 1
            p_start += gp

        if repeat > 1:
            loop_cm.__exit__(None, None, None)

    nc.finalize()
    return nc


_NC_CACHE = None


def _get_program():
    global _NC_CACHE
    if _NC_CACHE is None:
        _NC_CACHE = _build_program()
    return _NC_CACHE


def _make_in_maps(x, weight, bias, mask, gene_embedding):
    def kperm(a):  # (D, W) -> (KP, KT*W) with [d, k*W+w] = a[k*KP+d, w]
        w = a.shape[1]
        return np.ascontiguousarray(
            a.reshape(KT, KP, w).transpose(1, 0, 2).reshape(KP, KT * w))

    xT = x.T.reshape(KT, KP, B)                          # (16, 125, 64)
    in_maps = []
    for c in range(N_CORES):
        sl = slice(P_CORE * c, P_CORE * (c + 1))
        wT_c = weight[sl].T.reshape(KT, KP, P_CORE)
        mk_c = mask[:, sl].reshape(KT, KP, P_CORE)
        # k-major blocks [x_k | w_k | m_k] -> (125, 16*(64+32+32))
        xwm = np.ascontiguousarray(
            np.concatenate([xT, wT_c, mk_c], axis=2)
            .transpose(1, 0, 2).reshape(KP, -1))
        b_c = bias[sl]
        # (128, 16): col i = [bias[2i]]*64 ++ [bias[2i+1]]*64
        bias_sb = np.ascontiguousarray(
            np.repeat(b_c.reshape(P_CORE // 2, 2), B, axis=1).T)
        in_maps.append({"xwm": xwm, "g": gene_embedding, "bias": bias_sb})
    return in_maps


def kernel(x, weight, bias, mask, gene_embedding, _want_results=False, **_):
    x = np.ascontiguousarray(x, dtype=np.float32)
    weight = np.ascontiguousarray(weight, dtype=np.float32)
    bias = np.ascontiguousarray(bias, dtype=np.float32)
    mask = np.ascontiguousarray(mask, dtype=np.float32)
    g = np.ascontiguousarray(gene_embedding, dtype=np.float32)

    in_maps = _make_in_maps(x, weight, bias, mask, g)
    nc = _get_program()
    res = run_bass_kernel_spmd(nc, in_maps, list(range(N_CORES)))
    out = np.concatenate([r["out"] for r in res.results], axis=1)
    if _want_results:
        return out, res
    return out



# revision 4
# speedup vs baseline: 1.1257x; 1.1257x over previous
"""Trainium2 Bass kernel for nn_CustomizedLinear (masked pathway linear).

out[b, p*768+e] = sum_d x[b,d] * (weight*mask.T)[p,d] * G[d,e] + bias[p]
with B=64, P=256, D=2000, E=768.

Sharding: tensor-parallel over the pathway dim P — 32 pathways per core on
8 cores; x and gene_embedding replicated.

Per-core compute: fp8e4 matmuls in DoubleRow perf mode (0.5 PE cycles per
output column and 250 contraction rows per instruction = 4x fp32r MACs).
Accuracy is recovered with a 3-term split: G is host-split into fp8 hi+lo
(G ~ G_hi + G_lo), and the strip s = x*wm (scaled x256, folded into x on
host, exact) is device-split into fp8 s_hi + s_lo. psum accumulates
s_hi@G_hi + s_hi@G_lo + s_lo@G_hi; the dropped s_lo@G_lo term and fp8
residuals land at ~3e-3 fro overall.

Per DoubleRow k-tile (250 genes = 125 partitions x 2 slots, slot
innermost so every elementwise operand is 2-byte with stride-1 inner dim,
which enables the DVE 2x_1p mode for the bf16 strip multiply):
DVE computes t = x*wm (bf16), ACT casts s_hi = fp8(t), DVE/Pool compute
s_lo = fp8(t - s_hi). Pathways are processed in groups; each pair of
pathways packs M=128 stationary columns, G streams in two N=384 chunks.
Input/output DMAs are split across both HWDGE rings.
"""
import sys

sys.path.insert(0, "/opt/trn_rl_repo")

import numpy as np
import ml_dtypes
from contextlib import ExitStack

import concourse.bacc as bacc
import concourse.tile as tile
import concourse.mybir as mybir
from concourse.bass_utils import run_bass_kernel_spmd

F32 = mybir.dt.float32
BF16 = mybir.dt.bfloat16
F8 = mybir.dt.float8e4
E4M3 = ml_dtypes.float8_e4m3
BF = ml_dtypes.bfloat16

N_CORES = 8
B = 64          # batch
D = 2000        # genes (contraction)
E = 768         # embedding
P_TOT = 256     # pathways
P_CORE = P_TOT // N_CORES        # 32 pathways per core
KT = 8                           # DoubleRow k-tiles
S = 2                            # DoubleRow slots per partition
KP = D // (KT * S)               # 125 partitions per k-tile
NCH = 2                          # N chunks per pair
NC_N = E // NCH                  # 384
SCALE = 256.0                    # strip prescale (folded into x on host)
BLK = (B + 2 * P_CORE) * S       # 256 bf16 cols per k-tile [x|w|m]
GW = 2 * S * E                   # fp8 cols per k-tile in g [hi/lo, slot, e]


def _build_program(group_sizes=(4, 4, 4, 4, 4, 4, 4, 2, 2),
                   pool_subs=(1, 1, 1, 0, 1, 1, 0, 1), head_kt=2):
    assert sum(group_sizes) == P_CORE
    nc = bacc.Bacc()
    xwm_d = nc.declare_dram_parameter("xwm", [KP, KT * BLK], BF16,
                                      isOutput=False)
    g_d = nc.declare_dram_parameter("g", [KP, KT * GW], F8, isOutput=False)
    bias_d = nc.declare_dram_parameter("bias", [2 * B, P_CORE // 2], F32,
                                       isOutput=False)
    out_d = nc.declare_dram_parameter("out", [B, P_CORE * E], F32,
                                      isOutput=True)

    def ring(i):
        return nc.sync if i % 2 == 0 else nc.scalar

    with tile.TileContext(nc) as tc, ExitStack() as ctx:
        const = ctx.enter_context(tc.tile_pool(name="const", bufs=1))
        strips = ctx.enter_context(tc.tile_pool(name="strips", bufs=4))
        outs = ctx.enter_context(tc.tile_pool(name="outs", bufs=4))
        psum = ctx.enter_context(tc.tile_pool(name="psum", bufs=8,
                                              space="PSUM"))

        bias_t = const.tile([2 * B, P_CORE // 2], F32)
        nc.scalar.dma_start(out=bias_t[:], in_=bias_d[:])
        # x/w/m: head k-tiles now, tail after the first G chunks
        xwm_h = const.tile([KP, head_kt * BLK], BF16)
        nc.scalar.dma_start(out=xwm_h[:], in_=xwm_d[:, :head_kt * BLK])
        xwm_t = const.tile([KP, (KT - head_kt) * BLK], BF16)

        def xv(k):  # x view for k-tile: [KP, B, S]
            tl, kk = (xwm_h, k) if k < head_kt else (xwm_t, k - head_kt)
            return tl[:, kk * BLK:kk * BLK + B * S].rearrange(
                "d (b s) -> d b s", s=S)

        def wmv(tl, kk, n_kt):  # w/m views inside a block range
            w4 = tl[:].rearrange("d (k c) -> d k c", k=n_kt)[
                :, :, B * S:(B + P_CORE) * S].rearrange(
                "d k (p s) -> d k p s", s=S)
            m4 = tl[:].rearrange("d (k c) -> d k c", k=n_kt)[
                :, :, (B + P_CORE) * S:].rearrange(
                "d k (p s) -> d k p s", s=S)
            return w4, m4

        # masked weights, bf16: wm[d, k, p, s]
        wm_t = const.tile([KP, KT * P_CORE * S], BF16)
        wm4 = wm_t[:].rearrange("d (k p s) -> d k p s", k=KT, s=S)
        w4h, m4h = wmv(xwm_h, 0, head_kt)
        nc.vector.tensor_mul(wm4[:, :head_kt], w4h, m4h)

        # G stream: fp8 hi/lo interleaved per k-tile
        g_sb = const.tile([KP, KT * GW], F8)
        g5 = g_sb[:].rearrange("d (k h s e) -> d k h s e", k=KT, h=2, s=S)
        g_chunks = (1, 1, 2, 2, 2)
        k0 = 0
        for c, w in enumerate(g_chunks):
            eng = nc.sync if c % 2 == 0 else nc.scalar
            eng.dma_start(out=g_sb[:, k0 * GW:(k0 + w) * GW],
                          in_=g_d[:, k0 * GW:(k0 + w) * GW])
            if c == 0:
                # xwm tail + tail wm after the first G chunk is in flight
                nc.scalar.dma_start(out=xwm_t[:],
                                    in_=xwm_d[:, head_kt * BLK:])
                w4t, m4t = wmv(xwm_t, 0, KT - head_kt)
                nc.vector.tensor_mul(wm4[:, head_kt:], w4t, m4t)
            k0 += w
        assert k0 == KT

        out_p = out_d[:].rearrange("b (p e) -> p b e", p=P_CORE)

        odma = 0
        p_start = 0
        n_groups = len(group_sizes)
        for gi, gp in enumerate(group_sizes):
            npair = gp // 2
            ps = [psum.tile([2 * B, NC_N], F32, tag="ps",
                            name=f"ps{gi}_{i}") for i in range(npair * NCH)]
            for k in range(KT):
                fe = gp * B * S
                t = strips.tile([KP, fe], BF16, tag=f"t{gp}",
                                name=f"t{gi}_{k}")
                t4 = t[:].rearrange("d (p b s) -> d p b s", p=gp, s=S)
                x_bc = xv(k).unsqueeze(1).broadcast_to([KP, gp, B, S])
                w_bc = (wm4[:, k, p_start:p_start + gp]
                        .unsqueeze(2).broadcast_to([KP, gp, B, S]))
                nc.vector.tensor_mul(t4, x_bc, w_bc)
                # s_hi/s_lo written slot-major (the dual-fp8 ldweights ISA
                # requires contiguous stationary columns); t is read through
                # a strided view at no engine cost
                t4r = t[:].rearrange("d (p b s) -> d s p b", p=gp, s=S)
                s_hi = strips.tile([KP, fe], F8, tag=f"sh{gp}",
                                   name=f"sh{gi}_{k}")
                sh4 = s_hi[:].rearrange("d (s p b) -> d s p b", s=S, p=gp)
                nc.scalar.activation(sh4, t4r,
                                     mybir.ActivationFunctionType.Identity)
                s_lo = strips.tile([KP, fe], F8, tag=f"sl{gp}",
                                   name=f"sl{gi}_{k}")
                sl4 = s_lo[:].rearrange("d (s p b) -> d s p b", s=S, p=gp)
                sub_eng = nc.gpsimd if pool_subs[k] else nc.vector
                sub_eng.tensor_sub(sl4, t4r, sh4)

                hi3 = s_hi[:].rearrange("d (s m) -> d s m", s=S)
                lo3 = s_lo[:].rearrange("d (s m) -> d s m", s=S)
                for pr in range(npair):
                    lhs_hi = hi3[:, :, 128 * pr:128 * (pr + 1)]
                    lhs_lo = lo3[:, :, 128 * pr:128 * (pr + 1)]
                    for term, lhs, h in ((0, lhs_hi, 0), (1, lhs_hi, 1),
                                         (2, lhs_lo, 0)):
                        for n in range(NCH):
                            nc.tensor.matmul(
                                ps[NCH * pr + n][:],
                                lhs,
                                g5[:, k, h, :, NC_N * n:NC_N * (n + 1)],
                                start=(k == 0 and term == 0),
                                stop=(k == KT - 1 and term == 2),
                                perf_mode=mybir.MatmulPerfMode.DoubleRow,
                            )
            for pr in range(npair):
                pg = p_start // 2 + pr
                last = gi >= n_groups - 2
                o = outs.tile([2 * B, E], F32, tag="o", name=f"o{gi}_{pr}")
                p0 = 2 * pg
                for n in range(NCH):
                    nc.scalar.activation(
                        o[:, NC_N * n:NC_N * (n + 1)], ps[NCH * pr + n][:],
                        mybir.ActivationFunctionType.Identity,
                        bias=bias_t[:, pg:pg + 1], scale=1.0 / SCALE,
                    )
                    if last:
                        dst = out_p[p0:p0 + 2, :, NC_N * n:NC_N * (n + 1)]
                        ring(odma).dma_start(
                            out=dst, in_=o[:, NC_N * n:NC_N * (n + 1)])
                        odma += 1
                if not last:
                    ring(odma).dma_start(out=out_p[p0:p0 + 2, :, :], in_=o[:])
                    odma += 1
            p_start += gp

    nc.finalize()
    return nc


_NC_CACHE = None


def _get_program():
    global _NC_CACHE
    if _NC_CACHE is None:
        _NC_CACHE = _build_program()
    return _NC_CACHE


def _kpack(a):
    """[D, X] -> [KP, KT, S, X]: row d of k-tile k, slot s = gene k*250+s*125+d."""
    x = a.shape[1]
    return a.reshape(KT, S, KP, x).transpose(2, 0, 1, 3)


def _make_in_maps(x, weight, bias, mask, gene_embedding):
    # x scaled by 256 (exact in bf16), transposed to [D, B]
    xs = _kpack((x * SCALE).T.astype(BF))          # [KP, KT, S, B]
    xs = xs.transpose(0, 1, 3, 2)                  # [KP, KT, B, S]
    g32 = gene_embedding.astype(np.float32)
    g_hi = g32.astype(E4M3)
    g_lo = (g32 - g_hi.astype(np.float32)).astype(E4M3)
    gh = _kpack(g_hi)                              # [KP, KT, S, E]
    gl = _kpack(g_lo)
    g_pack = np.ascontiguousarray(
        np.stack([gh, gl], axis=2)                 # [KP, KT, 2, S, E]
    ).reshape(KP, KT * GW)

    in_maps = []
    for c in range(N_CORES):
        sl = slice(P_CORE * c, P_CORE * (c + 1))
        wp = _kpack(weight[sl].T.astype(BF)).transpose(0, 1, 3, 2)  # [KP,KT,P,S]
        mp = _kpack(mask[:, sl].astype(BF)).transpose(0, 1, 3, 2)
        xwm = np.ascontiguousarray(np.concatenate(
            [xs.reshape(KP, KT, B * S),
             wp.reshape(KP, KT, P_CORE * S),
             mp.reshape(KP, KT, P_CORE * S)], axis=2)).reshape(KP, KT * BLK)
        b_c = bias[sl]
        bias_sb = np.ascontiguousarray(
            np.repeat(b_c.reshape(P_CORE // 2, 2), B, axis=1).T.astype(
                np.float32))
        in_maps.append({"xwm": xwm, "g": g_pack, "bias": bias_sb})
    return in_maps


def kernel(x, weight, bias, mask, gene_embedding, _want_results=False, **_):
    x = np.ascontiguousarray(x, dtype=np.float32)
    weight = np.ascontiguousarray(weight, dtype=np.float32)
    bias = np.ascontiguousarray(bias, dtype=np.float32)
    mask = np.ascontiguousarray(mask, dtype=np.float32)
    g = np.ascontiguousarray(gene_embedding, dtype=np.float32)

    in_maps = _make_in_maps(x, weight, bias, mask, g)
    nc = _get_program()
    res = run_bass_kernel_spmd(nc, in_maps, list(range(N_CORES)))
    out = np.concatenate([r["out"] for r in res.results], axis=1)
    if _want_results:
        return out, res
    return out
